# revision 19
# baseline (speedup 1.0000x reference)
"""Trainium2 Bass kernel for nn_ConvexReLU.

Math: out[i,m] = sum_{j,k,l} G[j,k] * x[i,k,l] * (v-w)[j,l,m]

Reassociated as:
    d = v - w                              (host, elementwise)
    T[k,l,m]   = sum_j G[j,k] * d[j,l,m]   (device matmul, 68.7 GFLOP)
    out[i,m]   = sum_{k,l} x[i,k,l] * T[k,l,m]   (device matmul, 17.2 GFLOP)

Sharding: split l (in_dim, 256) across 8 cores (32 each). Each core computes
a full-shape (out_dim, batch) partial; host sums the 8 partials.

Device layout per core:
    g  : (1024 j, 1024 k)            full G, replicated
    d  : (1024 j, 32 l, 128 m)       l-shard of v-w
    xt : (8 pg, 128 p, 4*8*256)      l-shard of x, pre-tiled on host as
                                     [pg][k%128][dl][k//128][i]
    out: (128 m, 256 i)              partial of out^T

v2 changes vs the 156us baseline (all targeting non-matmul overhead; the
matmul stream itself is exact at the sustained PE clock: 512-col matmuls
issue every 216ns = 512 cols at 2.37GHz, zero per-instruction overhead):
  - 5 warmup matmuls on scratch SBUF right after the engine preamble: the
    PE p-state ramps (~1.2 -> 2.37 GHz over ~3us of activity) while the
    first g/d DMAs are still in flight, so real matmuls start near full
    clock (first real matmul at ~8.8us and ramped by ~11.8us, vs 10.6us
    and 16.1us for the baseline).
  - PSUM->SBUF T-copies split across vector and scalar engines (halves
    evacuation latency; removes the kgroup-transition bank-reuse stall).
  - x DMAs issued inside stage2 (one l-group before use), 4x524KB on the
    sync ring. Fusing them into one 2.1MB transfer, or issuing them any
    earlier, starves the periodic d chunks on the shared DMA engines and
    costs +5us of stage-1 stalls (measured).
Remaining overhead is framework-fixed: ~6.8us engine preamble + arg-table
loads before the first DMA can issue, and a ~5us teardown semaphore sweep
(S[3..207] zeroed one-by-one per engine) after the last matmul.
"""

import os
import sys

import numpy as np

for _p in ("/opt/trn_rl_repo", "/root/.axon_site/_ro/trn_rl_repo"):
    if os.path.isdir(_p) and _p not in sys.path:
        sys.path.insert(0, _p)

import concourse.bass as bass
import concourse.bacc as bacc
import concourse.mybir as mybir
from concourse.bass_utils import run_bass_kernel_spmd
from concourse.tile import TileContext

B, J, K, L, M = 256, 1024, 1024, 256, 128
NCORES = 8
LC = L // NCORES          # 32 l-values per core
NPG = 8                   # l-groups per core
LG = LC // NPG            # 4 l-values per group
NKT = K // 128            # 8 k-tiles
NJC = J // 128            # 8 j-chunks

F32 = mybir.dt.float32
F32R = mybir.dt.float32r
BF16 = mybir.dt.bfloat16

DTYPE = os.environ.get("BASS_KERNEL_DTYPE", "bf16")
N_WARMUP = int(os.environ.get("BASS_WARMUP_MMS", "5"))


def _dtypes(dtype_name: str):
    # (g/d stage-1 dtype, t/x stage-2 dtype). Stage-2 must be dtype-uniform:
    # f32r stationary + bf16 moving takes the explicit-LDWEIGHTS path, which
    # yields all-zero HW output for f32r weights.
    if dtype_name == "bf16":
        return BF16, BF16
    if dtype_name == "mixed":
        return F32R, BF16
    return F32R, F32R


def build_nc(dtype_name: str = DTYPE) -> bass.Bass:
    gd_dt, s2_dt = _dtypes(dtype_name)

    nc = bacc.Bacc(None, debug=False)

    g = nc.declare_dram_parameter("g", [J, K], gd_dt, isOutput=False)
    d = nc.declare_dram_parameter("d", [J, LC, M], gd_dt, isOutput=False)
    xt = nc.declare_dram_parameter(
        "xt", [NPG, 128, LG * NKT * B], s2_dt, isOutput=False
    )
    out = nc.declare_dram_parameter("out", [M, B], F32, isOutput=True)

    g_r = g.rearrange("(jc p) k -> p jc k", p=128)
    d_r = d.rearrange("(jc p) l m -> p jc (l m)", p=128)

    with TileContext(nc) as tc:
        with (
            tc.tile_pool(name="wpool", bufs=1) as wpool,
            tc.tile_pool(name="gpool", bufs=1) as gpool,
            tc.tile_pool(name="dpool", bufs=2) as dpool,
            tc.tile_pool(name="tpool", bufs=3) as tpool,
            tc.tile_pool(name="xpool", bufs=2) as xpool,
            tc.tile_pool(name="opool", bufs=1) as opool,
            tc.tile_pool(name="ps1", bufs=6, space="PSUM") as ps1,
            tc.tile_pool(name="pso", bufs=1, space="PSUM") as pso,
            tc.tile_pool(name="psw", bufs=1, space="PSUM") as psw,
        ):
            # ---- PE clock warmup: dummy matmuls with no DMA dependency ----
            # The tensor engine exits its preamble ~3.3us before the first
            # g/d DMA lands. Scratch matmuls in that window ramp the PE
            # p-state so the first real matmul runs at full clock.
            if N_WARMUP > 0:
                w_sb = wpool.tile([128, 512], s2_dt)
                nc.gpsimd.memset(w_sb[:], 0.0)
                warm_ps = psw.tile([128, 512], F32)
                for _ in range(N_WARMUP):
                    nc.tensor.matmul(
                        warm_ps[:],
                        w_sb[:, 0:128],
                        w_sb[:],
                        start=True,
                        stop=True,
                        skip_group_check=True,
                    )

            # per-jc DMAs so the first matmuls unblock after ~0.75 MB, not 6 MB.
            # pg=0's d chunks are interleaved with g chunks: stage-1 consumes
            # (g[jc], d[jc]) pairs in jc order.
            # g on the sync HWDGE ring, d on the scalar HWDGE ring: the two
            # rings dispatch in parallel (~650ns SP issue cost per dma_start).
            g_sb = gpool.tile([128, NJC, K], gd_dt)
            d_sb0 = dpool.tile([128, NJC, LG * M], gd_dt, tag="d")
            for jc in range(NJC):
                ga = nc.sync if jc % 2 == 0 else nc.scalar
                da = nc.scalar if jc % 2 == 0 else nc.sync
                if jc == 0:
                    # the first matmul's binding wait is d0[0]: split it
                    # across BOTH rings (64KB each) so it completes ~0.4us
                    # earlier than a single 128KB transfer on one ring
                    ga.dma_start(out=g_sb[:, 0, 0:256], in_=g_r[:, 0, 0:256])
                    da.dma_start(out=d_sb0[:, 0, 0:256], in_=d_r[:, 0, 0:256])
                    ga.dma_start(
                        out=d_sb0[:, 0, 256:], in_=d_r[:, 0, 256 : LG * M]
                    )
                    ga.dma_start(out=g_sb[:, 0, 256:], in_=g_r[:, 0, 256:])
                else:
                    ga.dma_start(out=g_sb[:, jc, :], in_=g_r[:, jc, :])
                    da.dma_start(
                        out=d_sb0[:, jc, :], in_=d_r[:, jc, 0 : LG * M]
                    )

            out_ps = pso.tile([M, B], F32)

            total_mm2 = NPG * LG * NKT
            # kt-groups per stage-1 pass: (6,2) so each jc chunk yields 6
            # back-to-back matmuls early on (outruns the g/d DMA cadence);
            # psum: 6 live stage-1 banks + 1 out bank + 1 warmup bank = 8
            KGROUPS = [(0, 6), (6, 2)]
            KH = 4  # stage-2 kt-group width

            mm2_state = [0]

            def stage2(pg, t_sb, x_sb):
                # out^T += T^T-slices @ x^T-slices for l-group pg.
                # x(pg)'s DMAs are issued HERE (one pg-period before the
                # matmuls execute): any earlier and they steal DMA
                # bandwidth from the startup-critical g/d chunks. Kept at
                # 4x524KB: a single fused 2.1MB transfer monopolizes the
                # ring's DMA-engine slices and starves the periodic d
                # chunks (measured +5us of stage-1 stalls).
                for dl in range(LG):
                    nc.sync.dma_start(
                        out=x_sb[:, dl], in_=xt[pg, :, dl * NKT * B : (dl + 1) * NKT * B]
                    )
                # kt-half outer: the first half's matmuls only need the
                # first half of the T copies, overlapping the second half.
                for half in range(NKT // KH):
                    for dl in range(LG):
                        for kt2 in range(KH):
                            kt = half * KH + kt2
                            nc.tensor.matmul(
                                out_ps[:],
                                t_sb[:, kt, dl * M : (dl + 1) * M],
                                x_sb[:, dl, kt, :],
                                start=(mm2_state[0] == 0),
                                stop=(mm2_state[0] == total_mm2 - 1),
                                skip_group_check=True,
                            )
                            mm2_state[0] += 1

            prev = None  # (pg, t_sb, x_sb) whose stage-2 is pending

            for pg in range(NPG):
                # ---- stage 1: T[k, (l,m)] for this l-group ----
                if pg == 0:
                    d_sb = d_sb0
                else:
                    # steady state: two half-DMAs per pg — coarse enough to
                    # amortize the dispatch, fine enough that the jc-outer
                    # loop can start on the first half
                    d_sb = dpool.tile([128, NJC, LG * M], gd_dt, tag="d")
                    nc.scalar.dma_start(
                        out=d_sb[:, 0 : NJC // 2, :],
                        in_=d_r[:, 0 : NJC // 2, pg * LG * M : (pg + 1) * LG * M],
                    )
                    nc.sync.dma_start(
                        out=d_sb[:, NJC // 2 :, :],
                        in_=d_r[:, NJC // 2 :, pg * LG * M : (pg + 1) * LG * M],
                    )

                t_sb = tpool.tile([128, NKT, LG * M], s2_dt, tag="t")
                for gi, (k0, kn) in enumerate(KGROUPS):
                    p1s = [ps1.tile([128, LG * M], F32, tag="p1", name=f"p1_{pg}_{gi}_{i}") for i in range(kn)]
                    # jc-outer: each (g[jc], d[jc]) pair is fully consumed as
                    # soon as its DMA lands -> PE starts ~3us into the kernel
                    for jc in range(NJC):
                        for kt2 in range(kn):
                            kt = k0 + kt2
                            nc.tensor.matmul(
                                p1s[kt2][:],
                                g_sb[:, jc, kt * 128 : (kt + 1) * 128],
                                d_sb[:, jc, :],
                                start=(jc == 0),
                                stop=(jc == NJC - 1),
                                skip_group_check=True,
                            )
                    # evacuate PSUM on both vector and scalar engines: two
                    # copies in flight halves the latency until the banks
                    # are reusable (and until stage-2 can consume T)
                    for kt2 in range(kn):
                        kt = k0 + kt2
                        ce = nc.vector if kt2 % 2 == 0 else nc.scalar
                        if ce is nc.vector:
                            ce.tensor_copy(out=t_sb[:, kt, :], in_=p1s[kt2][:])
                        else:
                            ce.copy(out=t_sb[:, kt, :], in_=p1s[kt2][:])

                x_sb = xpool.tile([128, LG, NKT, B], s2_dt, tag="x")

                # stage-2 lags stage-1 by one l-group: during the cold start
                # PE has two stage-1 passes (only g+d needed, ~4 MB) before
                # any x tile is required, hiding the initial DMA crunch.
                if prev is not None:
                    stage2(*prev)
                prev = (pg, t_sb, x_sb)

            stage2(*prev)

            out_sb = opool.tile([M, B], F32)
            nc.vector.tensor_copy(out=out_sb[:], in_=out_ps[:])
            nc.sync.dma_start(out=out[:], in_=out_sb[:])

    nc.finalize()
    return nc


_NC_CACHE: dict[str, bass.Bass] = {}


def _get_nc(dtype_name: str = DTYPE) -> bass.Bass:
    if dtype_name not in _NC_CACHE:
        _NC_CACHE[dtype_name] = build_nc(dtype_name)
    return _NC_CACHE[dtype_name]


def make_in_maps(x, G, v, w, dtype_name: str = DTYPE):
    x = np.asarray(x, dtype=np.float32)
    G = np.asarray(G, dtype=np.float32)
    v = np.asarray(v, dtype=np.float32)
    w = np.asarray(w, dtype=np.float32)

    d_full = v - w  # (J, L, M)

    import ml_dtypes

    if dtype_name == "bf16":
        gd_np, x_np = ml_dtypes.bfloat16, ml_dtypes.bfloat16
    elif dtype_name == "mixed":
        gd_np, x_np = np.float32, ml_dtypes.bfloat16
    else:
        gd_np, x_np = np.float32, np.float32

    G_io = np.ascontiguousarray(G.astype(gd_np))
    in_maps = []
    for c in range(NCORES):
        ls = slice(c * LC, (c + 1) * LC)
        d_c = np.ascontiguousarray(d_full[:, ls, :].astype(gd_np))
        # x (B,K,L) -> xt (NPG, 128, LG*NKT*B): xt[pg, p, dl, kt, i] =
        # x[i, kt*128+p, c*LC + pg*LG + dl]
        xc = x[:, :, ls].reshape(B, NKT, 128, NPG, LG)
        xt_c = np.ascontiguousarray(
            xc.transpose(3, 2, 4, 1, 0).reshape(NPG, 128, LG * NKT * B).astype(x_np)
        )
        in_maps.append({"g": G_io, "d": d_c, "xt": xt_c})
    return in_maps


def kernel(x, G, v, w):
    nc = _get_nc()
    in_maps = make_in_maps(x, G, v, w)
    res = run_bass_kernel_spmd(nc, in_maps, core_ids=list(range(NCORES)))
    acc = np.zeros((M, B), dtype=np.float64)
    for r in res.results:
        acc += r["out"].astype(np.float64)
    return np.ascontiguousarray(acc.T.astype(np.float32))
